# revision 1
# baseline (speedup 1.0000x reference)
"""DHT (discrete Hough transform) Trainium2 kernel — full pipeline.

out[bc, a, rho] = sum over 128-pixel chunks of X_chunk.T @ indicator
windows, accumulated in PSUM at per-(angle,chunk) column offsets via
per-element has_written semantics. Pixels chunked 8x16 (16x8 for the
transposed orientation) so each (angle, chunk) rho window is ~13 wide.
bf16 stationary (pixels on K partitions, bc on M) + bf16 indicator
stream as moving operand; fp32 PSUM accumulate.

Pipeline: passes of 8 angles on 4 PSUM banks (2 x 256-col regions per
bank), ping-ponging with DVE evacuation to SBUF staging and per-angle
DMA to the zero-initialized output. walrus --enable-ldw-opt=true dedups
the repeated per-chunk LDWEIGHTS.

Sync rules learned on HW:
 - PSUM readers must be gated by .then_inc on the LAST matmul (engine
   completion), never a sequencer sem_inc (fires before drain -> fatal
   PSUM collision).
 - Same for buffer recycling: DMA overwrite of an indicator segment is
   gated by .then_inc on the last matmul consuming it.
"""
import numpy as np
import ml_dtypes
import concourse.bass as bass
import concourse.mybir as mybir

NUM_ANGLE = 180
NUM_RHO = 180
H = 128
W_IMG = 128
P = 128
NCHUNK = 128
QROWS = 8
QCOLS = P // QROWS

BANKS_PER_PASS = 4
ANG_PER_PASS = 2 * BANKS_PER_PASS
SEG_COLS = 8192


def rho_table():
    irho = float(int(np.sqrt(H * H + W_IMG * W_IMG) + 1)) / float(NUM_RHO - 1)
    itheta = np.pi / NUM_ANGLE
    angles = np.arange(NUM_ANGLE, dtype=np.float64) * itheta
    tab_cos = np.cos(angles) / irho
    tab_sin = np.sin(angles) / irho
    xs = np.arange(W_IMG, dtype=np.float64) - (W_IMG // 2)
    ys = np.arange(H, dtype=np.float64) - (H // 2)
    r = np.round(xs[None, None, :] * tab_cos[:, None, None]
                 + ys[None, :, None] * tab_sin[:, None, None]).astype(np.int64)
    r = np.clip(r + NUM_RHO // 2, 0, NUM_RHO - 1)
    return r.astype(np.int32), tab_cos, tab_sin


def chunk_pixel_map(orient):
    perm = np.empty((NCHUNK, P), dtype=np.int64)
    nbx = W_IMG // QCOLS
    for k in range(NCHUNK):
        bi, bj = divmod(k, nbx)
        t = np.arange(QROWS)[:, None]
        u = np.arange(QCOLS)[None, :]
        yy = bi * QROWS + t
        xx = bj * QCOLS + u
        if orient == 0:
            perm[k] = (yy * W_IMG + xx).reshape(-1)
        else:
            perm[k] = (xx * W_IMG + yy).reshape(-1)
    return perm


class Plan:
    def __init__(self, passes_limit=None):
        r_tab, tab_cos, tab_sin = rho_table()
        self.r_flat = r_tab.reshape(NUM_ANGLE, H * W_IMG)
        self.orient = (np.abs(tab_cos) > np.abs(tab_sin)).astype(np.int64)
        self.perms = [chunk_pixel_map(0), chunk_pixel_map(1)]

        self.rho_ck = np.empty((NUM_ANGLE, NCHUNK, P), dtype=np.int64)
        for a in range(NUM_ANGLE):
            self.rho_ck[a] = self.r_flat[a][self.perms[self.orient[a]]]
        self.lo = self.rho_ck.min(axis=2)
        self.hi = self.rho_ck.max(axis=2)
        self.alo = self.lo.min(axis=1)
        self.ahi = self.hi.max(axis=1)
        self.awidth = self.ahi - self.alo + 1
        assert self.awidth.max() <= 256

        ax = [a for a in range(NUM_ANGLE) if self.orient[a] == 0]
        ay = [a for a in range(NUM_ANGLE) if self.orient[a] == 1]
        self.passes = []
        for group in (ax, ay):
            for i in range(0, len(group), ANG_PER_PASS):
                self.passes.append(group[i:i + ANG_PER_PASS])
        if passes_limit is not None:
            self.passes = self.passes[:passes_limit]
        self.npass = len(self.passes)

        self.win_w = (self.hi - self.lo + 1)

        # stream layout: blocks in (pass, chunk) order; segments cut at
        # block boundaries, <= SEG_COLS each
        self.segments = []
        self.block_start = {}
        self.block_pass = []  # per segment: last pass touching it
        col = 0
        seg_start = 0
        seg_last_pass = 0
        self.seg_end_pass = []
        for pi, angs in enumerate(self.passes):
            for k in range(NCHUNK):
                blk = int(sum(self.win_w[a, k] for a in angs))
                assert blk <= SEG_COLS
                if col + blk - seg_start > SEG_COLS:
                    self.segments.append((seg_start, col - seg_start))
                    self.seg_end_pass.append(seg_last_pass)
                    seg_start = col
                self.block_start[(pi, k)] = col
                seg_last_pass = pi
                col += blk
        self.segments.append((seg_start, col - seg_start))
        self.seg_end_pass.append(seg_last_pass)
        self.total_cols = col
        self.nseg = len(self.segments)

        # (pass, chunk) -> (segment, offset within segment)
        self.blk_seg = {}
        self.blk_off = {}
        si = 0
        for pi in range(self.npass):
            for k in range(NCHUNK):
                c = self.block_start[(pi, k)]
                while c >= self.segments[si][0] + self.segments[si][1]:
                    si += 1
                self.blk_seg[(pi, k)] = si
                self.blk_off[(pi, k)] = c - self.segments[si][0]

    def build_indicators(self, scale=1.0):
        ind = np.zeros((P, self.total_cols), dtype=ml_dtypes.bfloat16)
        for pi, angs in enumerate(self.passes):
            for k in range(NCHUNK):
                c = self.block_start[(pi, k)]
                for a in angs:
                    lo = self.lo[a, k]
                    w = int(self.win_w[a, k])
                    rr = self.rho_ck[a, k]
                    ind[np.arange(P), c + (rr - lo)] = scale
                    c += w
        return ind

    def build_x_layouts(self, x_bc):
        out = []
        for o in (0, 1):
            pm = self.perms[o].reshape(-1)
            xl = x_bc[:, pm].T.astype(ml_dtypes.bfloat16)
            out.append(np.ascontiguousarray(xl))
        return out


def build_kernel(plan: Plan, bc: int = 128):
    nc = bass.Bass("TRN2")
    xa_d = nc.dram_tensor("xa", [NCHUNK * P, bc], mybir.dt.bfloat16, kind="ExternalInput")
    xb_d = nc.dram_tensor("xb", [NCHUNK * P, bc], mybir.dt.bfloat16, kind="ExternalInput")
    ind_d = nc.dram_tensor("ind", [P, plan.total_cols], mybir.dt.bfloat16, kind="ExternalInput")
    out_d = nc.dram_tensor("out", [NUM_ANGLE, bc, NUM_RHO], mybir.dt.float32, kind="ExternalOutput")

    nseg = plan.nseg
    total_angles = sum(len(p) for p in plan.passes)

    with (
        nc.sbuf_tensor("xa_t", [P, NCHUNK * bc], mybir.dt.bfloat16) as xa_t,
        nc.sbuf_tensor("xb_t", [P, NCHUNK * bc], mybir.dt.bfloat16) as xb_t,
        nc.sbuf_tensor("ind_t", [P, 2 * SEG_COLS], mybir.dt.bfloat16) as ind_t,
        nc.sbuf_tensor("zero_t", [1, 512], mybir.dt.bfloat16) as zero_t,
        nc.sbuf_tensor("one_t", [1, P], mybir.dt.bfloat16) as one_t,
        nc.sbuf_tensor("stage_t", [P, 2 * ANG_PER_PASS * NUM_RHO], mybir.dt.float32) as stage_t,
        nc.psum_tensor("acc", [P, 8 * 512], mybir.dt.float32) as acc,
        nc.semaphore("x_sem") as x_sem,
        nc.semaphore("ind_sem0") as ind_sem0,
        nc.semaphore("ind_sem1") as ind_sem1,
        nc.semaphore("pe_prog") as pe_prog,
        nc.semaphore("pe_pass") as pe_pass,
        nc.semaphore("evac_sem") as evac_sem,
        nc.semaphore("stage_sem") as stage_sem,
        nc.semaphore("outdma0") as outdma0,
        nc.semaphore("outdma1") as outdma1,
        nc.Block() as block,
    ):
        @block.sync
        def _(sync):
            xa_v = xa_d[:].rearrange("(k p) b -> p k b", p=P)
            xb_v = xb_d[:].rearrange("(k p) b -> p k b", p=P)
            sync.dma_start(xa_t[:].rearrange("p (k b) -> p k b", k=NCHUNK), xa_v).then_inc(x_sem, 16)
            sync.dma_start(xb_t[:].rearrange("p (k b) -> p k b", k=NCHUNK), xb_v).then_inc(x_sem, 16)

            # interleave segment loads and out-DMAs in unblock order
            actions = []
            for s in range(nseg):
                unblock = -1 if s < 2 else plan.seg_end_pass[s - 2]
                actions.append(((unblock, 0, s), ("seg", s)))
            n = 0
            for pi, angs in enumerate(plan.passes):
                for j, a in enumerate(angs):
                    n += 1
                    actions.append(((pi, 1, n), ("out", pi, j, a, n)))
            actions.sort(key=lambda t: t[0])
            for _, act in actions:
                if act[0] == "seg":
                    s = act[1]
                    if s >= 2:
                        sync.wait_ge(pe_prog, s - 1)
                    s0, ncols = plan.segments[s]
                    sync.dma_start(
                        ind_t[:, (s % 2) * SEG_COLS:(s % 2) * SEG_COLS + ncols],
                        ind_d[:, s0:s0 + ncols],
                    ).then_inc(ind_sem0 if s % 2 == 0 else ind_sem1, 16)
                else:
                    _, pi, j, a, n = act
                    sync.wait_ge(stage_sem, n)
                    w = int(plan.awidth[a])
                    st = (pi % 2) * ANG_PER_PASS * NUM_RHO + j * NUM_RHO
                    sync.dma_start(
                        out_d[a, :, int(plan.alo[a]):int(plan.alo[a]) + w],
                        stage_t[:, st:st + w],
                    ).then_inc(outdma0 if pi % 2 == 0 else outdma1, 16)
            n_even = sum(len(p) for q, p in enumerate(plan.passes) if q % 2 == 0)
            n_odd = total_angles - n_even
            if n_even:
                sync.wait_ge(outdma0, 16 * n_even)
            if n_odd:
                sync.wait_ge(outdma1, 16 * n_odd)

        @block.vector
        def _(vector):
            vector.memset(zero_t[:], 0.0).then_inc(evac_sem, 1)
            vector.memset(one_t[:], 1.0).then_inc(evac_sem, 1)
            for pi, angs in enumerate(plan.passes):
                vector.wait_ge(pe_pass, pi + 1)
                if pi >= 2:
                    need = 16 * sum(len(plan.passes[q]) for q in range(pi - 1)
                                    if q % 2 == pi % 2)
                    vector.wait_ge(outdma0 if pi % 2 == 0 else outdma1, need)
                bank0 = 0 if pi % 2 == 0 else BANKS_PER_PASS
                for j, a in enumerate(angs):
                    w = int(plan.awidth[a])
                    reg = (bank0 + j // 2) * 512 + (j % 2) * 256
                    st = (pi % 2) * ANG_PER_PASS * NUM_RHO + j * NUM_RHO
                    vector.tensor_copy(
                        stage_t[:, st:st + w], acc[:, reg:reg + w]
                    ).then_inc(stage_sem, 1)

        @block.tensor
        def _(tensor):
            tensor.wait_ge(x_sem, 32)
            cur_seg = -1
            prev_mm = None
            for pi, angs in enumerate(plan.passes):
                if pi == 0:
                    tensor.wait_ge(evac_sem, 2)
                if pi >= 2:
                    need = sum(len(plan.passes[q]) for q in range(pi - 1))
                    tensor.wait_ge(stage_sem, need)
                bank0 = 0 if pi % 2 == 0 else BANKS_PER_PASS
                for b in range(BANKS_PER_PASS):
                    tensor.matmul(
                        acc[:, (bank0 + b) * 512:(bank0 + b) * 512 + 512],
                        lhsT=one_t[:1, :],
                        rhs=zero_t[:1, :],
                        start=True, stop=False, skip_group_check=True,
                    )
                x_t = xa_t if plan.orient[angs[0]] == 0 else xb_t
                last_mm = None
                for k in range(NCHUNK):
                    s = plan.blk_seg[(pi, k)]
                    if s != cur_seg:
                        if cur_seg >= 0:
                            prev_mm.then_inc(pe_prog, 1)
                        cur_seg = s
                        tensor.wait_ge(ind_sem0 if s % 2 == 0 else ind_sem1,
                                       16 * (s // 2 + 1))
                    off = (s % 2) * SEG_COLS + plan.blk_off[(pi, k)]
                    lhs = x_t[:, k * bc:(k + 1) * bc]
                    for j, a in enumerate(angs):
                        w = int(plan.win_w[a, k])
                        reg = (bank0 + j // 2) * 512 + (j % 2) * 256
                        col = reg + int(plan.lo[a, k] - plan.alo[a])
                        last_mm = tensor.matmul(
                            acc[:, col:col + w],
                            lhsT=lhs,
                            rhs=ind_t[:, off:off + w],
                            start=False, stop=False, skip_group_check=True,
                        )
                        prev_mm = last_mm
                        off += w
                tensor.drain().then_inc(pe_pass, 1)

    return nc


_PLAN = None


def get_plan():
    global _PLAN
    if _PLAN is None:
        _PLAN = Plan()
    return _PLAN


def enable_ldw_opt():
    import concourse.bass_utils as bu
    if getattr(bu, "_ldw_patched", False):
        return
    orig_run = bu.run_command

    def patched(argv, **kw):
        argv = [a.replace("--enable-ldw-opt=false", "--enable-ldw-opt=true")
                for a in argv]
        return orig_run(argv, **kw)

    bu.run_command = patched
    bu._ldw_patched = True


def run_dht(x, n_cores=8, trace=False, ldw_opt=True):
    """x: [B, C, H, W] fp32 -> ([B, C, A, R] fp32, BassKernelResults)."""
    import concourse.bass_utils as bu
    if ldw_opt:
        enable_ldw_opt()

    plan = get_plan()
    B, C, hh, ww = x.shape
    bc_total = B * C
    bc = bc_total // n_cores
    xf = np.asarray(x, dtype=np.float32).reshape(bc_total, hh * ww)
    ind = plan.build_indicators()

    nc = build_kernel(plan, bc=bc)
    in_maps = []
    for c in range(n_cores):
        xa, xb = plan.build_x_layouts(xf[c * bc:(c + 1) * bc])
        in_maps.append({"xa": xa, "xb": xb, "ind": ind})

    res = bu.run_bass_kernel_spmd(nc, in_maps, core_ids=list(range(n_cores)),
                                  trace=trace)
    outs = [res.results[c]["out"] for c in range(n_cores)]
    full = np.stack(outs, axis=0)            # [n_cores, A, bc, R]
    full = full.transpose(0, 2, 1, 3)        # [n_cores, bc, A, R]
    full = full.reshape(B, C, NUM_ANGLE, NUM_RHO)
    return full, res


# ----------------------------------------------------------------- entry
B_FULL, C_FULL = 8, 128
N_CORES = 8


def kernel(x):
    """Full DHT: x [8,128,128,128] fp32 -> [8,128,180,180] fp32."""
    x = np.asarray(x, dtype=np.float32)
    out, _ = run_dht(x, n_cores=N_CORES, trace=False, ldw_opt=True)
    return out.astype(np.float32)


# revision 2
# speedup vs baseline: 1.0498x; 1.0498x over previous
"""DHT (discrete Hough transform) Trainium2 kernel — full pipeline.

out[bc, a, rho] = sum over 128-pixel chunks of X_chunk.T @ indicator
windows, accumulated in PSUM at per-(angle,chunk) column offsets via
per-element has_written semantics. Pixels chunked 8x16 (16x8 for the
transposed orientation) so each (angle, chunk) rho window is ~13 wide.
bf16 stationary (pixels on K partitions, bc on M) + bf16 indicator
stream as moving operand; fp32 PSUM accumulate.

Pipeline: passes of 8 angles on 4 PSUM banks (2 x 256-col regions per
bank), ping-ponging with DVE evacuation to SBUF staging and per-angle
DMA to the zero-initialized output. walrus --enable-ldw-opt=true dedups
the repeated per-chunk LDWEIGHTS.

Sync rules learned on HW:
 - PSUM readers must be gated by .then_inc on the LAST matmul (engine
   completion), never a sequencer sem_inc (fires before drain -> fatal
   PSUM collision).
 - Same for buffer recycling: DMA overwrite of an indicator segment is
   gated by .then_inc on the last matmul consuming it.
"""
import numpy as np
import ml_dtypes
import concourse.bass as bass
import concourse.mybir as mybir
from concourse.ap import AP

PSUM_STRIDE = 4096
IND_NP_DT = ml_dtypes.float8_e4m3


NUM_ANGLE = 180
NUM_RHO = 180
H = 128
W_IMG = 128
P = 128
NCHUNK = 128
QROWS = 8
QCOLS = P // QROWS

BANKS_PER_PASS = 4
ANG_PER_PASS = 2 * BANKS_PER_PASS
SEG_COLS = 8192


def rho_table():
    irho = float(int(np.sqrt(H * H + W_IMG * W_IMG) + 1)) / float(NUM_RHO - 1)
    itheta = np.pi / NUM_ANGLE
    angles = np.arange(NUM_ANGLE, dtype=np.float64) * itheta
    tab_cos = np.cos(angles) / irho
    tab_sin = np.sin(angles) / irho
    xs = np.arange(W_IMG, dtype=np.float64) - (W_IMG // 2)
    ys = np.arange(H, dtype=np.float64) - (H // 2)
    r = np.round(xs[None, None, :] * tab_cos[:, None, None]
                 + ys[None, :, None] * tab_sin[:, None, None]).astype(np.int64)
    r = np.clip(r + NUM_RHO // 2, 0, NUM_RHO - 1)
    return r.astype(np.int32), tab_cos, tab_sin


def chunk_pixel_map(orient):
    perm = np.empty((NCHUNK, P), dtype=np.int64)
    nbx = W_IMG // QCOLS
    for k in range(NCHUNK):
        bi, bj = divmod(k, nbx)
        t = np.arange(QROWS)[:, None]
        u = np.arange(QCOLS)[None, :]
        yy = bi * QROWS + t
        xx = bj * QCOLS + u
        if orient == 0:
            perm[k] = (yy * W_IMG + xx).reshape(-1)
        else:
            perm[k] = (xx * W_IMG + yy).reshape(-1)
    return perm


class Plan:
    def __init__(self, passes_limit=None):
        r_tab, tab_cos, tab_sin = rho_table()
        self.r_flat = r_tab.reshape(NUM_ANGLE, H * W_IMG)
        self.orient = (np.abs(tab_cos) > np.abs(tab_sin)).astype(np.int64)
        self.perms = [chunk_pixel_map(0), chunk_pixel_map(1)]

        self.rho_ck = np.empty((NUM_ANGLE, NCHUNK, P), dtype=np.int64)
        for a in range(NUM_ANGLE):
            self.rho_ck[a] = self.r_flat[a][self.perms[self.orient[a]]]
        self.lo = self.rho_ck.min(axis=2)
        self.hi = self.rho_ck.max(axis=2)
        self.alo = self.lo.min(axis=1)
        self.ahi = self.hi.max(axis=1)
        self.awidth = self.ahi - self.alo + 1
        assert self.awidth.max() <= 256

        ax = [a for a in range(NUM_ANGLE) if self.orient[a] == 0]
        ay = [a for a in range(NUM_ANGLE) if self.orient[a] == 1]
        self.passes = []
        for group in (ax, ay):
            for i in range(0, len(group), ANG_PER_PASS):
                self.passes.append(group[i:i + ANG_PER_PASS])
        if passes_limit is not None:
            self.passes = self.passes[:passes_limit]
        self.npass = len(self.passes)

        self.win_w = (self.hi - self.lo + 1)

        # pair structure per pass: banks get angle pairs (j, j+1); a tail
        # pass with odd count ends with a single
        self.pass_groups = []  # per pass: list of ('pair',(a1,a2)) / ('one',(a,))
        for angs in self.passes:
            groups = []
            i = 0
            while i + 1 < len(angs):
                groups.append(("pair", (angs[i], angs[i + 1])))
                i += 2
            if i < len(angs):
                groups.append(("one", (angs[i],)))
            self.pass_groups.append(groups)

        # stream layout: blocks in (pass, chunk) order; segments cut at
        # block boundaries, <= SEG_COLS each
        self.segments = []
        self.block_start = {}
        self.block_pass = []  # per segment: last pass touching it
        col = 0
        seg_start = 0
        seg_last_pass = 0
        self.seg_end_pass = []
        for pi, angs in enumerate(self.passes):
            groups = self.pass_groups[pi]
            for k in range(NCHUNK):
                blk = 0
                for kind, aa in groups:
                    if kind == "pair":
                        blk += 2 * int(max(self.win_w[aa[0], k], self.win_w[aa[1], k]))
                    else:
                        blk += int(self.win_w[aa[0], k])
                assert blk <= SEG_COLS
                if col + blk - seg_start > SEG_COLS:
                    self.segments.append((seg_start, col - seg_start))
                    self.seg_end_pass.append(seg_last_pass)
                    seg_start = col
                self.block_start[(pi, k)] = col
                seg_last_pass = pi
                col += blk
        self.segments.append((seg_start, col - seg_start))
        self.seg_end_pass.append(seg_last_pass)
        self.total_cols = col
        self.nseg = len(self.segments)

        # (pass, chunk) -> (segment, offset within segment)
        self.blk_seg = {}
        self.blk_off = {}
        si = 0
        for pi in range(self.npass):
            for k in range(NCHUNK):
                c = self.block_start[(pi, k)]
                while c >= self.segments[si][0] + self.segments[si][1]:
                    si += 1
                self.blk_seg[(pi, k)] = si
                self.blk_off[(pi, k)] = c - self.segments[si][0]

    def build_indicators(self, scale=1.0):
        ind = np.zeros((P, self.total_cols), dtype=IND_NP_DT)
        pr = np.arange(P)
        for pi, groups in enumerate(self.pass_groups):
            for k in range(NCHUNK):
                c = self.block_start[(pi, k)]
                for kind, aa in groups:
                    if kind == "pair":
                        a1, a2 = aa
                        wmax = int(max(self.win_w[a1, k], self.win_w[a2, k]))
                        ind[pr, c + (self.rho_ck[a1, k] - self.lo[a1, k])] = scale
                        ind[pr, c + wmax + (self.rho_ck[a2, k] - self.lo[a2, k])] = scale
                        c += 2 * wmax
                    else:
                        a = aa[0]
                        ind[pr, c + (self.rho_ck[a, k] - self.lo[a, k])] = scale
                        c += int(self.win_w[a, k])
        return ind

    def build_x_layouts(self, x_bc):
        out = []
        for o in (0, 1):
            pm = self.perms[o].reshape(-1)
            xl = x_bc[:, pm].T.astype(ml_dtypes.bfloat16)
            out.append(np.ascontiguousarray(xl))
        return out


def build_kernel(plan: Plan, bc: int = 128):
    nc = bass.Bass("TRN2")
    xa_d = nc.dram_tensor("xa", [NCHUNK * P, bc], mybir.dt.bfloat16, kind="ExternalInput")
    xb_d = nc.dram_tensor("xb", [NCHUNK * P, bc], mybir.dt.bfloat16, kind="ExternalInput")
    ind_d = nc.dram_tensor("ind", [P, plan.total_cols], mybir.dt.float8e4, kind="ExternalInput")
    out_d = nc.dram_tensor("out", [NUM_ANGLE, bc, NUM_RHO], mybir.dt.float32, kind="ExternalOutput")

    nseg = plan.nseg
    total_angles = sum(len(p) for p in plan.passes)

    with (
        nc.sbuf_tensor("xa_t", [P, NCHUNK * bc], mybir.dt.bfloat16) as xa_t,
        nc.sbuf_tensor("xb_t", [P, NCHUNK * bc], mybir.dt.bfloat16) as xb_t,
        nc.sbuf_tensor("ind_t", [P, 2 * SEG_COLS], mybir.dt.float8e4) as ind_t,
        nc.sbuf_tensor("zero_t", [1, 512], mybir.dt.bfloat16) as zero_t,
        nc.sbuf_tensor("one_t", [1, P], mybir.dt.bfloat16) as one_t,
        nc.sbuf_tensor("stage_t", [P, 2 * ANG_PER_PASS * NUM_RHO], mybir.dt.float32) as stage_t,
        nc.psum_tensor("acc", [P, 8 * 512], mybir.dt.float32) as acc,
        nc.semaphore("x_sem") as x_sem,
        nc.semaphore("ind_sem0") as ind_sem0,
        nc.semaphore("ind_sem1") as ind_sem1,
        nc.semaphore("pe_prog") as pe_prog,
        nc.semaphore("pe_pass") as pe_pass,
        nc.semaphore("evac_sem") as evac_sem,
        nc.semaphore("stage_sem") as stage_sem,
        nc.semaphore("outdma0") as outdma0,
        nc.semaphore("outdma1") as outdma1,
        nc.Block() as block,
    ):
        @block.sync
        def _(sync):
            xa_v = xa_d[:].rearrange("(k p) b -> p k b", p=P)
            xb_v = xb_d[:].rearrange("(k p) b -> p k b", p=P)
            sync.dma_start(xa_t[:].rearrange("p (k b) -> p k b", k=NCHUNK), xa_v).then_inc(x_sem, 16)
            sync.dma_start(xb_t[:].rearrange("p (k b) -> p k b", k=NCHUNK), xb_v).then_inc(x_sem, 16)

            # interleave segment loads and out-DMAs in unblock order
            actions = []
            for s in range(nseg):
                unblock = -1 if s < 2 else plan.seg_end_pass[s - 2]
                actions.append(((unblock, 0, s), ("seg", s)))
            n = 0
            for pi, angs in enumerate(plan.passes):
                for j, a in enumerate(angs):
                    n += 1
                    actions.append(((pi, 1, n), ("out", pi, j, a, n)))
            actions.sort(key=lambda t: t[0])
            for _, act in actions:
                if act[0] == "seg":
                    s = act[1]
                    if s >= 2:
                        sync.wait_ge(pe_prog, s - 1)
                    s0, ncols = plan.segments[s]
                    sync.dma_start(
                        ind_t[:, (s % 2) * SEG_COLS:(s % 2) * SEG_COLS + ncols],
                        ind_d[:, s0:s0 + ncols],
                    ).then_inc(ind_sem0 if s % 2 == 0 else ind_sem1, 16)
                else:
                    _, pi, j, a, n = act
                    sync.wait_ge(stage_sem, n)
                    w = int(plan.awidth[a])
                    st = (pi % 2) * ANG_PER_PASS * NUM_RHO + j * NUM_RHO
                    sync.dma_start(
                        out_d[a, :, int(plan.alo[a]):int(plan.alo[a]) + w],
                        stage_t[:, st:st + w],
                    ).then_inc(outdma0 if pi % 2 == 0 else outdma1, 16)
            n_even = sum(len(p) for q, p in enumerate(plan.passes) if q % 2 == 0)
            n_odd = total_angles - n_even
            if n_even:
                sync.wait_ge(outdma0, 16 * n_even)
            if n_odd:
                sync.wait_ge(outdma1, 16 * n_odd)

        @block.vector
        def _(vector):
            vector.memset(zero_t[:], 0.0).then_inc(evac_sem, 1)
            vector.memset(one_t[:], 1.0).then_inc(evac_sem, 1)
            for pi, angs in enumerate(plan.passes):
                vector.wait_ge(pe_pass, pi + 1)
                if pi >= 2:
                    need = 16 * sum(len(plan.passes[q]) for q in range(pi - 1)
                                    if q % 2 == pi % 2)
                    vector.wait_ge(outdma0 if pi % 2 == 0 else outdma1, need)
                bank0 = 0 if pi % 2 == 0 else BANKS_PER_PASS
                for j, a in enumerate(angs):
                    w = int(plan.awidth[a])
                    reg = (bank0 + j // 2) * 512 + (j % 2) * 256
                    st = (pi % 2) * ANG_PER_PASS * NUM_RHO + j * NUM_RHO
                    vector.tensor_copy(
                        stage_t[:, st:st + w], acc[:, reg:reg + w]
                    ).then_inc(stage_sem, 1)

        @block.tensor
        def _(tensor):
            tensor.wait_ge(x_sem, 32)
            cur_seg = -1
            prev_mm = None
            for pi, angs in enumerate(plan.passes):
                if pi == 0:
                    tensor.wait_ge(evac_sem, 2)
                if pi >= 2:
                    need = sum(len(plan.passes[q]) for q in range(pi - 1))
                    tensor.wait_ge(stage_sem, need)
                bank0 = 0 if pi % 2 == 0 else BANKS_PER_PASS
                for b in range(BANKS_PER_PASS):
                    tensor.matmul(
                        acc[:, (bank0 + b) * 512:(bank0 + b) * 512 + 512],
                        lhsT=one_t[:1, :],
                        rhs=zero_t[:1, :],
                        start=True, stop=False, skip_group_check=True,
                    )
                x_t = xa_t if plan.orient[angs[0]] == 0 else xb_t
                last_mm = None
                for k in range(NCHUNK):
                    s = plan.blk_seg[(pi, k)]
                    if s != cur_seg:
                        if cur_seg >= 0:
                            prev_mm.then_inc(pe_prog, 1)
                        cur_seg = s
                        tensor.wait_ge(ind_sem0 if s % 2 == 0 else ind_sem1,
                                       16 * (s // 2 + 1))
                    off = (s % 2) * SEG_COLS + plan.blk_off[(pi, k)]
                    lhs = x_t[:, k * bc:(k + 1) * bc]
                    for gi, (kind, aa) in enumerate(plan.pass_groups[pi]):
                        bankbase = (bank0 + gi) * 512
                        if kind == "pair":
                            a1, a2 = aa
                            w1 = int(plan.win_w[a1, k]); w2 = int(plan.win_w[a2, k])
                            wmax = max(w1, w2)
                            m1 = bankbase + int(plan.lo[a1, k] - plan.alo[a1])
                            m2 = bankbase + 256 + int(plan.lo[a2, k] - plan.alo[a2])
                            out_ap = AP(acc[:].tensor, m1,
                                        [[PSUM_STRIDE, P], [m2 - m1, 2], [1, wmax]])
                            last_mm = tensor.matmul(
                                out_ap, lhsT=lhs,
                                rhs=ind_t[:, off:off + 2 * wmax],
                                start=False, stop=False, skip_group_check=True,
                            )
                            off += 2 * wmax
                        else:
                            a = aa[0]
                            w = int(plan.win_w[a, k])
                            col = bankbase + int(plan.lo[a, k] - plan.alo[a])
                            last_mm = tensor.matmul(
                                acc[:, col:col + w], lhsT=lhs,
                                rhs=ind_t[:, off:off + w],
                                start=False, stop=False, skip_group_check=True,
                            )
                            off += w
                        prev_mm = last_mm
                tensor.drain().then_inc(pe_pass, 1)

    return nc


_PLAN = None


def get_plan():
    global _PLAN
    if _PLAN is None:
        _PLAN = Plan()
    return _PLAN


def enable_ldw_opt():
    import concourse.bass_utils as bu
    if getattr(bu, "_ldw_patched", False):
        return
    orig_run = bu.run_command

    def patched(argv, **kw):
        argv = [a.replace("--enable-ldw-opt=false", "--enable-ldw-opt=true")
                for a in argv]
        return orig_run(argv, **kw)

    bu.run_command = patched
    bu._ldw_patched = True


def run_dht(x, n_cores=8, trace=False, ldw_opt=True):
    """x: [B, C, H, W] fp32 -> ([B, C, A, R] fp32, BassKernelResults)."""
    import concourse.bass_utils as bu
    if ldw_opt:
        enable_ldw_opt()

    plan = get_plan()
    B, C, hh, ww = x.shape
    bc_total = B * C
    bc = bc_total // n_cores
    xf = np.asarray(x, dtype=np.float32).reshape(bc_total, hh * ww)
    ind = plan.build_indicators()

    nc = build_kernel(plan, bc=bc)
    in_maps = []
    for c in range(n_cores):
        xa, xb = plan.build_x_layouts(xf[c * bc:(c + 1) * bc])
        in_maps.append({"xa": xa, "xb": xb, "ind": ind})

    res = bu.run_bass_kernel_spmd(nc, in_maps, core_ids=list(range(n_cores)),
                                  trace=trace)
    outs = [res.results[c]["out"] for c in range(n_cores)]
    full = np.stack(outs, axis=0)            # [n_cores, A, bc, R]
    full = full.transpose(0, 2, 1, 3)        # [n_cores, bc, A, R]
    full = full.reshape(B, C, NUM_ANGLE, NUM_RHO)
    return full, res


# ----------------------------------------------------------------- entry
N_CORES = 8


def kernel(x):
    """Full DHT: x [8,128,128,128] fp32 -> [8,128,180,180] fp32."""
    x = np.asarray(x, dtype=np.float32)
    out, _ = run_dht(x, n_cores=N_CORES, trace=False, ldw_opt=True)
    return out.astype(np.float32)
